# revision 21
# baseline (speedup 1.0000x reference)
"""Trainium2 Bass kernel for nn_MHC (dense transformer block: QKV -> causal
attention -> conv1d(k=3) -> causal attention (same K/V) -> out proj).

Sharding over 8 NeuronCores: data-parallel on batch (2) x tensor-parallel on
heads (16 heads -> 4 per core). Cores 0-3 own batch 0, cores 4-7 batch 1.
Per-token-block AllGather (groups of 4) exchanges attention-1 context so each
core can run the channel-mixing conv for its own output channels; gathers are
pipelined behind attention compute (blocks processed in descending order so
the smallest block is the serial tail).

All matmul operands are bf16 (fp32 PSUM accumulation); softmax denominators
are batched into one fast-approx reciprocal per block; the two heads of a
pair issue their K=64 score matmuls back-to-back into disjoint PSUM banks so
the PE runs them concurrently (row-tiled).
"""

import numpy as np
import ml_dtypes

import concourse.bacc as bacc
import concourse.mybir as mybir
import concourse.tile as tile
from concourse.bass import ts
from concourse.bass_utils import run_bass_kernel_spmd

# Problem shapes (hardcoded per contract)
B, S, D = 2, 2048, 1024
H, DH = 16, 64
N_CORES = 8
HPC = 4          # heads per core
CL = HPC * DH    # 256 local channels
KT = D // 128    # 8 k-tiles over the model dim
NJ = S // 512    # 4 t-blocks of 512
NS = S // 128    # 16 s-tiles of 128
GROUPS = [[0, 1, 2, 3], [4, 5, 6, 7]]

F32 = mybir.dt.float32
F32R = mybir.dt.float32r
BF16 = mybir.dt.bfloat16
EXP = mybir.ActivationFunctionType.Exp
MULT = mybir.AluOpType.mult
ADD = mybir.AluOpType.add

_CACHE = {}


def _r(ap):
    """View an f32 AP as float32r for PE matmuls."""
    return ap.bitcast(F32R)


def build_kernel(collective=True):
    key = ("nc", collective)
    if key in _CACHE:
        return _CACHE[key]
    nc = bacc.Bacc("TRN2", target_bir_lowering=False, debug=False,
                   num_devices=N_CORES if collective else 1)

    # ---- I/O ----
    xT_d = nc.dram_tensor("xT", [D, S], BF16, kind="ExternalInput")
    wqk_d = nc.dram_tensor("wqk", [D, 4 * 128], BF16, kind="ExternalInput")
    wv_d = nc.dram_tensor("wv", [D, CL], BF16, kind="ExternalInput")
    qkb_d = nc.dram_tensor("qkb", [4, 128], F32, kind="ExternalInput")
    vbb_d = nc.dram_tensor("vbb", [128, CL], F32, kind="ExternalInput")
    cw_d = nc.dram_tensor("cw", [3, D, CL], BF16, kind="ExternalInput")
    cb_d = nc.dram_tensor("cb", [2, 128], F32, kind="ExternalInput")
    ow_d = nc.dram_tensor("ow", [CL, D], BF16, kind="ExternalInput")
    tri2_d = nc.dram_tensor("tri2", [128, 256], BF16, kind="ExternalInput")
    on1_d = nc.dram_tensor("on1", [1, 64], BF16, kind="ExternalInput")
    outT_d = nc.dram_tensor("outT", [D, S], BF16, kind="ExternalOutput")

    xT_v = xT_d.ap().rearrange("(kt p) t -> p kt t", p=128)
    outT_v = outT_d.ap().rearrange("(m p) t -> p m t", p=128)

    with tile.TileContext(nc) as tc:
        with (
            tc.tile_pool(name="w", bufs=1) as wp,
            tc.tile_pool(name="big", bufs=1) as bigp,
            tc.tile_pool(name="xs", bufs=2) as xsp,
            tc.tile_pool(name="p", bufs=3) as pp,
            tc.tile_pool(name="raw", bufs=8) as rawp,
            tc.tile_pool(name="den", bufs=2) as denp,
            tc.tile_pool(name="blk", bufs=2) as blkp,
            tc.tile_pool(name="ob", bufs=3) as obp,
            tc.tile_pool(name="ps", bufs=1, space="PSUM") as psp,
            tc.tile_pool(name="dram", bufs=1, space="DRAM") as dramp,
        ):
            # ---- load weights / constants ----
            # wqk + the first x block lead the DMA queue so QKV starts ASAP
            wqk = wp.tile([128, KT, 512], BF16)
            nc.sync.dma_start(wqk[:], wqk_d.ap().rearrange("(kt p) m -> p kt m", p=128))
            xt0 = xsp.tile([128, KT, 512], BF16, tag="xt")
            nc.sync.dma_start(xt0[:], xT_v[:, :, ts(0, 512)])
            wv = wp.tile([128, KT, CL], BF16)
            nc.sync.dma_start(wv[:], wv_d.ap().rearrange("(kt p) c -> p kt c", p=128))
            qkb = wp.tile([128, 4], F32)
            nc.sync.dma_start(qkb[:], qkb_d.ap().rearrange("m p -> p m"))
            vbb = wp.tile([128, CL], F32)
            nc.sync.dma_start(vbb[:], vbb_d.ap())
            tri2 = wp.tile([128, 2, 128], BF16)
            nc.sync.dma_start(tri2[:], tri2_d.ap().rearrange("p (h t) -> p h t", h=2))
            ones = wp.tile([1, 64], BF16)
            nc.sync.dma_start(ones[:], on1_d.ap())
            # conv / out-proj weights are DMA'd later (just before phase 2)
            # so the first xt block isn't queued behind them
            cw = wp.tile([128, 3, KT, CL], BF16)
            cb = wp.tile([128, 2], F32)
            ow = wp.tile([128, 2, 8, 128], BF16)

            # ---- persistent activations ----
            qpair = bigp.tile([128, 2, S], BF16, name="qpair")
            kpair = bigp.tile([128, 2, S], BF16, name="kpair")
            q2pair = bigp.tile([128, 2, S], BF16, name="q2pair")
            v_sb = bigp.tile([128, NS, HPC, 65], BF16, name="v_sb")
            nc.vector.memset(v_sb[:, :, :, 64:65], 1.0)
            ctxg = bigp.tile([128, KT, S + 2], BF16, name="ctxg")
            nc.vector.memset(ctxg[:, :, 0:1], 0.0)
            nc.vector.memset(ctxg[:, :, S + 1:S + 2], 0.0)

            cc_in = [dramp.tile([CL, 512], BF16, tag=f"ci{j}", name=f"ci{j}")
                     for j in range(NJ)]
            cc_out = [dramp.tile([D, 512], BF16, tag=f"co{j}", name=f"co{j}")
                      for j in range(NJ)]

            # ================= Phase A: QKV projections =================
            for j in range(NJ):
                if j == 0:
                    xt = xt0
                else:
                    xt = xsp.tile([128, KT, 512], BF16, tag="xt")
                    nc.sync.dma_start(xt[:], xT_v[:, :, ts(j, 512)])
                # q (m=0,1) / k (m=2,3) transposed: [channels, t]
                for m in range(4):
                    ps = psp.tile([128, 512], F32, tag="mm", bufs=2)
                    for kt in range(KT):
                        nc.tensor.matmul(ps[:], wqk[:, kt, ts(m, 128)],
                                         xt[:, kt, :],
                                         start=(kt == 0), stop=(kt == KT - 1))
                    dstq = qpair if m < 2 else kpair
                    nc.vector.tensor_scalar(dstq[:, m % 2, ts(j, 512)], ps[:],
                                            qkb[:, m:m + 1], None, ADD)
                # v token-major: [t, c] for the 4 s-tiles of this block
                for u in range(4):
                    ps = psp.tile([128, CL], F32, tag="mm", bufs=2)
                    for kt in range(KT):
                        nc.tensor.matmul(ps[:], xt[:, kt, ts(u, 128)],
                                         wv[:, kt, :],
                                         start=(kt == 0), stop=(kt == KT - 1))
                    st_i = 4 * j + u
                    nc.vector.tensor_tensor(
                        v_sb[:, st_i, :, 0:64],
                        ps.rearrange("p (h e) -> p h e", e=64),
                        vbb.rearrange("p (h e) -> p h e", e=64), ADD)

            # ================= Attention block (used twice) =================
            def attn_block(qsrc, j):
                """Scores + ctx accumulation for t-block j, both head pairs.
                Returns (raws, den): raws[kp][hh] = bf16 [64,512] unnormalized
                ctx; den = [4,512] f32 softmax denominators."""
                den = denp.tile([1, 4, 512], F32, tag="den")
                raws = [[None, None], [None, None]]
                i_last = 4 * j + 3
                def expctx(st_v, kp, cps, i, c0):
                    """softmax numerator + ctx accumulation for s-tile i."""
                    p = pp.tile([128, 2, 512], BF16, tag="p")
                    nc.scalar.activation(p[:, :, c0:512],
                                         st_v[:, :, c0:512], EXP)
                    if i - 4 * j >= 0:
                        nc.vector.tensor_tensor(p[:, :, c0:c0 + 128],
                                                p[:, :, c0:c0 + 128],
                                                tri2[:], MULT)
                    for hh in range(2):
                        nc.tensor.matmul(cps[hh][0:65, c0:512],
                                         v_sb[:, i, 2 * kp + hh, :],
                                         p[:, hh, c0:512],
                                         start=(i == 0), stop=(i == i_last))

                pend_i = None
                for kp in range(2):
                    cps = [psp.tile([128, 512], F32, tag="ctx", name="ctx",
                                    bufs=2) for _ in range(2)]
                    for i in range(4 * j + 4):
                        r = i - 4 * j
                        # columns below the diagonal tile are fully masked:
                        # compute only cols [c0, 512) of this t-block
                        c0 = 128 * r if r > 0 else 0
                        st = psp.tile([128, 1024], F32, tag="st", bufs=2)
                        st_v = st.rearrange("p (h t) -> p h t", h=2)
                        # both heads' score matmuls back-to-back: disjoint
                        # row groups (K=64 at partitions 0/64) + disjoint
                        # PSUM banks -> concurrent on the PE
                        for hh in range(2):
                            row = slice(64 * hh, 64 * hh + 64)
                            nc.tensor.matmul(st[:, 512 * hh + c0:512 * (hh + 1)],
                                             kpair[row, kp, ts(i, 128)],
                                             qsrc[row, kp,
                                                  j * 512 + c0:(j + 1) * 512])
                        # exp/ctx of the PREVIOUS s-tile: its exp overlaps
                        # this tile's score matmuls instead of stalling PE
                        if pend_i is not None:
                            expctx(*pend_i)
                        pend_i = (st_v, kp, cps, i, c0)
                    expctx(*pend_i)
                    pend_i = None
                    # evacuate PSUM promptly: denominators + raw ctx to SBUF
                    for hh in range(2):
                        ri = 2 * kp + hh
                        nc.scalar.copy(den[:, ri, :], cps[hh][64:65, :])
                        raw = rawp.tile([64, 512], BF16, tag="raw")
                        nc.scalar.copy(raw[:], cps[hh][0:64, :])
                        raws[kp][hh] = raw
                return raws, den

            def normalize(raws, den, dst_blk):
                """dst_blk[128, 2, 512] bf16 = raws / den (softmax divide)."""
                rc32 = denp.tile([1, 4, 512], F32, tag="rc32")
                nc.vector.reciprocal_approx_fast(
                    rc32.rearrange("p a t -> p (a t)"),
                    den.rearrange("p a t -> p (a t)"))
                rc = denp.tile([1, 4, 512], BF16, tag="rc")
                nc.vector.tensor_copy(
                    out=rc.rearrange("p a t -> p (a t)"),
                    in_=rc32.rearrange("p a t -> p (a t)"))
                for kp in range(2):
                    for hh in range(2):
                        ri = 2 * kp + hh
                        bc = psp.tile([64, 512], F32, tag="mm", bufs=2)
                        nc.tensor.matmul(bc[:], ones[:], rc[:, ri, :])
                        nc.vector.tensor_tensor(
                            dst_blk[64 * hh:64 * hh + 64, kp, :],
                            raws[kp][hh][:], bc[:], MULT)

            # ---- attention 1 (descending blocks, pipelined AllGather) ----
            pend = None
            for j in reversed(range(NJ)):
                raws, den = attn_block(qpair, j)
                if pend is not None:
                    pend()

                def mk1(raws=raws, den=den, j=j):
                    def go():
                        blk = blkp.tile([128, 2, 512], BF16, tag="c1")
                        normalize(raws, den, blk)
                        nc.sync.dma_start(
                            cc_in[j].opt().rearrange("(k p) t -> p k t", p=128),
                            blk[:])
                        if collective:
                            nc.gpsimd.collective_compute(
                                "AllGather", mybir.AluOpType.bypass,
                                replica_groups=GROUPS,
                                ins=[cc_in[j].opt()], outs=[cc_out[j].opt()])
                        else:
                            for g4 in range(4):
                                nc.sync.dma_start(
                                    cc_out[j].opt()[CL * g4:CL * (g4 + 1), :],
                                    cc_in[j].opt()[:])
                        nc.sync.dma_start(
                            ctxg[:, :, 1 + j * 512:1 + (j + 1) * 512],
                            cc_out[j].opt().rearrange("(kt p) t -> p kt t",
                                                      p=128))
                    return go
                pend = mk1()
            pend()

            # ---- conv1d(k=3) -> q2, attention 2, out projection ----
            nc.sync.dma_start(cw[:], cw_d.ap().rearrange("a (kt p) o -> p a kt o", p=128))
            nc.sync.dma_start(cb[:], cb_d.ap().rearrange("m p -> p m"))
            nc.sync.dma_start(
                ow[:], ow_d.ap().rearrange("(kt p) (m q) -> p kt m q", p=128, q=128))
            pend = None
            for j in reversed(range(NJ)):
                # conv for t-block j (needs gathered blocks j-1, j, j+1)
                for ot in range(2):
                    ps = psp.tile([128, 512], F32, tag="mm", bufs=2)
                    first = True
                    for kt in range(KT):
                        for tap in range(3):
                            nc.tensor.matmul(
                                ps[:], cw[:, tap, kt, ts(ot, 128)],
                                ctxg[:, kt, j * 512 + tap: j * 512 + tap + 512],
                                start=first,
                                stop=(kt == KT - 1 and tap == 2))
                            first = False
                    nc.vector.tensor_scalar(q2pair[:, ot, ts(j, 512)], ps[:],
                                            cb[:, ot:ot + 1], None, ADD)
                raws, den = attn_block(q2pair, j)
                if pend is not None:
                    pend()

                def mk2(raws=raws, den=den, j=j):
                    def go():
                        blk = blkp.tile([128, 2, 512], BF16, tag="c2")
                        normalize(raws, den, blk)
                        for m in range(8):
                            ps = psp.tile([128, 512], F32, tag="mm", bufs=2)
                            for kt in range(2):
                                nc.tensor.matmul(ps[:], ow[:, kt, m, :],
                                                 blk[:, kt, :],
                                                 start=(kt == 0),
                                                 stop=(kt == 1))
                            ob = obp.tile([128, 512], BF16, tag="ob")
                            nc.vector.tensor_copy(out=ob[:], in_=ps[:])
                            nc.sync.dma_start(outT_v[:, m, ts(j, 512)], ob[:])
                    return go
                pend = mk2()
            pend()

    nc.compile()
    _CACHE[key] = nc
    return nc


def prep_inputs(x, Wqkv_w, Wqkv_b, conv_w, conv_b, out_w, out_b):
    """Build the 8 per-core input maps from the full problem inputs."""
    x = np.asarray(x, np.float32)
    Wqkv_w = np.asarray(Wqkv_w, np.float32)
    Wqkv_b = np.asarray(Wqkv_b, np.float32)
    conv_w = np.asarray(conv_w, np.float32)
    conv_b = np.asarray(conv_b, np.float32)
    out_w = np.asarray(out_w, np.float32)

    scale = 1.0 / np.sqrt(DH).astype(np.float32)
    tri = (np.arange(128)[None, :] >= np.arange(128)[:, None]).astype(np.float32)
    tri2 = np.concatenate([tri, tri], axis=1).astype(ml_dtypes.bfloat16)

    in_maps = []
    for g in range(N_CORES):
        b, hg = g // 4, g % 4
        h0 = HPC * hg
        # q/k row blocks, m-tiles: [q pair0, q pair1, k pair0, k pair1]
        rows = []
        biases = []
        for blk, sc in ((0, scale), (1, 1.0)):
            for pr in range(2):
                r0 = blk * D + (h0 + 2 * pr) * DH
                rows.append(Wqkv_w[r0:r0 + 128, :] * sc)
                biases.append(Wqkv_b[r0:r0 + 128] * sc)
        wqk = np.ascontiguousarray(
            np.concatenate(rows, axis=0).T).astype(ml_dtypes.bfloat16)
        qkb = np.stack(biases, axis=0)  # [4, 128]
        c0 = CL * hg
        wv = np.ascontiguousarray(
            Wqkv_w[2 * D + c0:2 * D + c0 + CL, :].T).astype(ml_dtypes.bfloat16)
        vbb = np.ascontiguousarray(
            np.broadcast_to(Wqkv_b[2 * D + c0:2 * D + c0 + CL], (128, CL)))
        cw = np.ascontiguousarray(
            (conv_w[c0:c0 + CL, :, :] * scale).transpose(2, 1, 0)
        ).astype(ml_dtypes.bfloat16)  # [3, D, CL]
        cb = (conv_b[c0:c0 + CL] * scale).reshape(2, 128).astype(np.float32)
        owm = np.ascontiguousarray(
            out_w[:, c0:c0 + CL].T).astype(ml_dtypes.bfloat16)  # [CL, D]
        in_maps.append({
            "xT": np.ascontiguousarray(x[b].T).astype(ml_dtypes.bfloat16),
            "wqk": wqk, "wv": wv,
            "qkb": np.ascontiguousarray(qkb),
            "vbb": vbb, "cw": cw,
            "cb": np.ascontiguousarray(cb),
            "ow": owm, "tri2": tri2,
            "on1": np.ones((1, 64), ml_dtypes.bfloat16),
        })
    return in_maps


def postprocess(results, out_b):
    out_b = np.asarray(out_b, np.float32)
    out = np.empty((B, S, D), np.float32)
    for b in range(B):
        acc = np.zeros((D, S), np.float64)
        for g in GROUPS[b]:
            acc += np.asarray(results[g]["outT"], np.float64)
        out[b] = acc.T.astype(np.float32) + out_b[None, :]
    return out


def kernel(x, Wqkv_w, Wqkv_b, conv_w, conv_b, out_w, out_b):
    nc = build_kernel()
    in_maps = prep_inputs(x, Wqkv_w, Wqkv_b, conv_w, conv_b, out_w, out_b)
    res = run_bass_kernel_spmd(nc, in_maps, core_ids=list(range(N_CORES)))
    return postprocess(res.results, out_b)


# revision 29
# speedup vs baseline: 1.0787x; 1.0787x over previous
"""Trainium2 Bass kernel for nn_MHC (dense transformer block: QKV -> causal
attention -> conv1d(k=3) -> causal attention (same K/V) -> out proj).

Sharding over 8 NeuronCores: data-parallel on batch (2) x tensor-parallel on
heads (16 heads -> 4 per core). Cores 0-3 own batch 0, cores 4-7 batch 1.
Per-token-block AllGather (groups of 4) exchanges attention-1 context so each
core can run the channel-mixing conv for its own output channels; gathers are
pipelined behind attention compute (blocks processed in descending order so
the smallest block is the serial tail).

All matmul operands are bf16 (fp32 PSUM accumulation); softmax denominators
are batched into one fast-approx reciprocal per block; the two heads of a
pair issue their K=64 score matmuls back-to-back into disjoint PSUM banks so
the PE runs them concurrently (row-tiled).
"""

import numpy as np
import ml_dtypes

import concourse.bacc as bacc
import concourse.mybir as mybir
import concourse.tile as tile
from concourse.bass import ts
from concourse.bass_utils import run_bass_kernel_spmd

# Problem shapes (hardcoded per contract)
B, S, D = 2, 2048, 1024
H, DH = 16, 64
N_CORES = 8
HPC = 4          # heads per core
CL = HPC * DH    # 256 local channels
KT = D // 128    # 8 k-tiles over the model dim
NJ = S // 512    # 4 t-blocks of 512
NS = S // 128    # 16 s-tiles of 128
GROUPS = [[0, 1, 2, 3], [4, 5, 6, 7]]

F32 = mybir.dt.float32
F32R = mybir.dt.float32r
BF16 = mybir.dt.bfloat16
EXP = mybir.ActivationFunctionType.Exp
MULT = mybir.AluOpType.mult
ADD = mybir.AluOpType.add

_CACHE = {}


def _r(ap):
    """View an f32 AP as float32r for PE matmuls."""
    return ap.bitcast(F32R)


def build_kernel(collective=True):
    key = ("nc", collective)
    if key in _CACHE:
        return _CACHE[key]
    nc = bacc.Bacc("TRN2", target_bir_lowering=False, debug=False,
                   num_devices=N_CORES if collective else 1)

    # ---- I/O ----
    xT_d = nc.dram_tensor("xT", [D, S], BF16, kind="ExternalInput")
    wqk_d = nc.dram_tensor("wqk", [D, 4 * 128], BF16, kind="ExternalInput")
    wv_d = nc.dram_tensor("wv", [D, CL], BF16, kind="ExternalInput")
    qkb_d = nc.dram_tensor("qkb", [4, 128], F32, kind="ExternalInput")
    vbb_d = nc.dram_tensor("vbb", [128, CL], F32, kind="ExternalInput")
    cw_d = nc.dram_tensor("cw", [3, D, CL], BF16, kind="ExternalInput")
    cb_d = nc.dram_tensor("cb", [2, 128], F32, kind="ExternalInput")
    ow_d = nc.dram_tensor("ow", [CL, D], BF16, kind="ExternalInput")
    tri2_d = nc.dram_tensor("tri2", [128, 256], BF16, kind="ExternalInput")
    on1_d = nc.dram_tensor("on1", [1, 64], BF16, kind="ExternalInput")
    outT_d = nc.dram_tensor("outT", [D, S], BF16, kind="ExternalOutput")

    xT_v = xT_d.ap().rearrange("(kt p) t -> p kt t", p=128)
    outT_v = outT_d.ap().rearrange("(m p) t -> p m t", p=128)

    with tile.TileContext(nc) as tc:
        with (
            tc.tile_pool(name="w", bufs=1) as wp,
            tc.tile_pool(name="big", bufs=1) as bigp,
            tc.tile_pool(name="xs", bufs=2) as xsp,
            tc.tile_pool(name="p", bufs=3) as pp,
            tc.tile_pool(name="raw", bufs=8) as rawp,
            tc.tile_pool(name="den", bufs=1) as denp,
            tc.tile_pool(name="blk", bufs=2) as blkp,
            tc.tile_pool(name="ob", bufs=3) as obp,
            tc.tile_pool(name="ps", bufs=1, space="PSUM") as psp,
            tc.tile_pool(name="dram", bufs=1, space="DRAM") as dramp,
        ):
            # ---- load weights / constants ----
            # wqk + the first x block lead the DMA queue so QKV starts ASAP
            wqk = wp.tile([128, KT, 512], BF16)
            nc.sync.dma_start(wqk[:], wqk_d.ap().rearrange("(kt p) m -> p kt m", p=128))
            xt0 = xsp.tile([128, KT, 512], BF16, tag="xt0", bufs=1)
            nc.sync.dma_start(xt0[:], xT_v[:, :, ts(0, 512)])
            wv = wp.tile([128, KT, CL], BF16)
            nc.sync.dma_start(wv[:], wv_d.ap().rearrange("(kt p) c -> p kt c", p=128))
            qkb = wp.tile([128, 4], F32)
            nc.sync.dma_start(qkb[:], qkb_d.ap().rearrange("m p -> p m"))
            vbb = wp.tile([128, CL], F32)
            nc.sync.dma_start(vbb[:], vbb_d.ap())
            tri2 = wp.tile([128, 2, 128], BF16)
            nc.sync.dma_start(tri2[:], tri2_d.ap().rearrange("p (h t) -> p h t", h=2))
            ones = wp.tile([1, 64], BF16)
            nc.sync.dma_start(ones[:], on1_d.ap())
            # conv / out-proj weights are DMA'd later (just before phase 2)
            # so the first xt block isn't queued behind them
            cw = wp.tile([128, 3, KT, CL], BF16)
            cb = wp.tile([128, 2], F32)
            ow = wp.tile([128, 2, 8, 128], BF16)

            # ---- persistent activations ----
            qpair = bigp.tile([128, 2, S], BF16, name="qpair")
            kpair = bigp.tile([128, 2, S], BF16, name="kpair")
            q2pair = bigp.tile([128, 2, S], BF16, name="q2pair")
            v_sb = bigp.tile([128, NS, HPC, 65], BF16, name="v_sb")
            nc.vector.memset(v_sb[:, :, :, 64:65], 1.0)
            ctxg = bigp.tile([128, KT, S + 2], BF16, name="ctxg")
            nc.vector.memset(ctxg[:, :, 0:1], 0.0)
            nc.vector.memset(ctxg[:, :, S + 1:S + 2], 0.0)

            cc_in = [dramp.tile([CL, 512], BF16, tag=f"ci{j}", name=f"ci{j}")
                     for j in range(NJ)]
            cc_out = [dramp.tile([D, 512], BF16, tag=f"co{j}", name=f"co{j}")
                      for j in range(NJ)]

            # ================= Phase A: K/V projections =================
            # (q is projected per-block inside the attention-1 loop — the
            # dense 16-MM q chains break up attention's PE micro-idle
            # pattern that trips the activity throttle)
            xts = []
            for j in range(NJ):
                if j == 0:
                    xt = xt0
                else:
                    xt = xsp.tile([128, KT, 512], BF16, tag=f"xt{j}", bufs=1)
                    nc.sync.dma_start(xt[:], xT_v[:, :, ts(j, 512)])
                xts.append(xt)
                # k (m=2,3) transposed: [channels, t]
                for m in range(2, 4):
                    ps = psp.tile([128, 512], F32, tag="mm", bufs=2)
                    for kt in range(KT):
                        nc.tensor.matmul(ps[:], wqk[:, kt, ts(m, 128)],
                                         xt[:, kt, :],
                                         start=(kt == 0), stop=(kt == KT - 1))
                    nc.vector.tensor_scalar(kpair[:, m % 2, ts(j, 512)], ps[:],
                                            qkb[:, m:m + 1], None, ADD)
                # v token-major: [t, c] for the 4 s-tiles of this block
                for u in range(4):
                    ps = psp.tile([128, CL], F32, tag="mm", bufs=2)
                    for kt in range(KT):
                        nc.tensor.matmul(ps[:], xt[:, kt, ts(u, 128)],
                                         wv[:, kt, :],
                                         start=(kt == 0), stop=(kt == KT - 1))
                    st_i = 4 * j + u
                    nc.vector.tensor_tensor(
                        v_sb[:, st_i, :, 0:64],
                        ps.rearrange("p (h e) -> p h e", e=64),
                        vbb.rearrange("p (h e) -> p h e", e=64), ADD)

            # ================= Attention block (used twice) =================
            def attn_block(qsrc, j):
                """Scores + ctx accumulation for t-block j, both head pairs.
                Returns (raws, den): raws[kp][hh] = bf16 [64,512] unnormalized
                ctx; den = [4,512] f32 softmax denominators."""
                den = denp.tile([1, 4, 512], F32, tag="den", bufs=2)
                raws = [[None, None], [None, None]]
                i_last = 4 * j + 3
                def expctx(st_v, kp, cps, i, c0):
                    """softmax numerator + ctx accumulation for s-tile i."""
                    p = pp.tile([128, 2, 512], BF16, tag="p")
                    nc.scalar.activation(p[:, :, c0:512],
                                         st_v[:, :, c0:512], EXP)
                    if i - 4 * j >= 0:
                        nc.vector.tensor_tensor(p[:, :, c0:c0 + 128],
                                                p[:, :, c0:c0 + 128],
                                                tri2[:], MULT)
                    for hh in range(2):
                        nc.tensor.matmul(cps[hh][0:65, c0:512],
                                         v_sb[:, i, 2 * kp + hh, :],
                                         p[:, hh, c0:512],
                                         start=(i == 0), stop=(i == i_last))

                pend_i = None
                for kp in range(2):
                    cps = [psp.tile([128, 512], F32, tag="ctx", name="ctx",
                                    bufs=2) for _ in range(2)]
                    for i in range(4 * j + 4):
                        r = i - 4 * j
                        # columns below the diagonal tile are fully masked:
                        # compute only cols [c0, 512) of this t-block
                        c0 = 128 * r if r > 0 else 0
                        st = psp.tile([128, 1024], F32, tag="st", bufs=2)
                        st_v = st.rearrange("p (h t) -> p h t", h=2)
                        # both heads' score matmuls back-to-back: disjoint
                        # row groups (K=64 at partitions 0/64) + disjoint
                        # PSUM banks -> concurrent on the PE
                        for hh in range(2):
                            row = slice(64 * hh, 64 * hh + 64)
                            nc.tensor.matmul(st[:, 512 * hh + c0:512 * (hh + 1)],
                                             kpair[row, kp, ts(i, 128)],
                                             qsrc[row, kp,
                                                  j * 512 + c0:(j + 1) * 512])
                        # exp/ctx of the PREVIOUS s-tile: its exp overlaps
                        # this tile's score matmuls instead of stalling PE
                        if pend_i is not None:
                            expctx(*pend_i)
                        pend_i = (st_v, kp, cps, i, c0)
                    expctx(*pend_i)
                    pend_i = None
                    # evacuate PSUM promptly: denominators + raw ctx to SBUF
                    for hh in range(2):
                        ri = 2 * kp + hh
                        nc.scalar.copy(den[:, ri, :], cps[hh][64:65, :])
                        raw = rawp.tile([64, 512], BF16, tag="raw")
                        nc.scalar.copy(raw[:], cps[hh][0:64, :])
                        raws[kp][hh] = raw
                return raws, den

            def normalize(raws, den, dst_blk):
                """dst_blk[128, 2, 512] bf16 = raws / den (softmax divide)."""
                rc32 = denp.tile([1, 4, 512], F32, tag="rc32", bufs=1)
                nc.vector.reciprocal_approx_fast(
                    rc32.rearrange("p a t -> p (a t)"),
                    den.rearrange("p a t -> p (a t)"))
                rc = denp.tile([1, 4, 512], BF16, tag="rc", bufs=1)
                nc.vector.tensor_copy(
                    out=rc.rearrange("p a t -> p (a t)"),
                    in_=rc32.rearrange("p a t -> p (a t)"))
                for kp in range(2):
                    for hh in range(2):
                        ri = 2 * kp + hh
                        bc = psp.tile([64, 512], F32, tag="mm", bufs=2)
                        nc.tensor.matmul(bc[:], ones[:], rc[:, ri, :])
                        nc.vector.tensor_tensor(
                            dst_blk[64 * hh:64 * hh + 64, kp, :],
                            raws[kp][hh][:], bc[:], MULT)

            # ---- attention 1 (descending blocks, pipelined AllGather) ----
            pend = None
            for j in reversed(range(NJ)):
                # q projection for this block
                for m in range(2):
                    ps = psp.tile([128, 512], F32, tag="mm", bufs=2)
                    for kt in range(KT):
                        nc.tensor.matmul(ps[:], wqk[:, kt, ts(m, 128)],
                                         xts[j][:, kt, :],
                                         start=(kt == 0), stop=(kt == KT - 1))
                    nc.vector.tensor_scalar(qpair[:, m, ts(j, 512)], ps[:],
                                            qkb[:, m:m + 1], None, ADD)
                raws, den = attn_block(qpair, j)
                if pend is not None:
                    pend()

                def mk1(raws=raws, den=den, j=j):
                    def go():
                        blk = blkp.tile([128, 2, 512], BF16, tag="c1")
                        normalize(raws, den, blk)
                        nc.sync.dma_start(
                            cc_in[j].opt().rearrange("(k p) t -> p k t", p=128),
                            blk[:])
                        if collective:
                            nc.gpsimd.collective_compute(
                                "AllGather", mybir.AluOpType.bypass,
                                replica_groups=GROUPS,
                                ins=[cc_in[j].opt()], outs=[cc_out[j].opt()])
                        else:
                            for g4 in range(4):
                                nc.sync.dma_start(
                                    cc_out[j].opt()[CL * g4:CL * (g4 + 1), :],
                                    cc_in[j].opt()[:])
                        nc.sync.dma_start(
                            ctxg[:, :, 1 + j * 512:1 + (j + 1) * 512],
                            cc_out[j].opt().rearrange("(kt p) t -> p kt t",
                                                      p=128))
                    return go
                pend = mk1()
            pend()

            # ---- conv1d(k=3) -> q2, attention 2, out projection ----
            nc.sync.dma_start(cw[:], cw_d.ap().rearrange("a (kt p) o -> p a kt o", p=128))
            nc.sync.dma_start(cb[:], cb_d.ap().rearrange("m p -> p m"))
            nc.sync.dma_start(
                ow[:], ow_d.ap().rearrange("(kt p) (m q) -> p kt m q", p=128, q=128))
            pend = None
            for j in reversed(range(NJ)):
                # conv for t-block j (needs gathered blocks j-1, j, j+1)
                for ot in range(2):
                    ps = psp.tile([128, 512], F32, tag="mm", bufs=2)
                    first = True
                    for kt in range(KT):
                        for tap in range(3):
                            nc.tensor.matmul(
                                ps[:], cw[:, tap, kt, ts(ot, 128)],
                                ctxg[:, kt, j * 512 + tap: j * 512 + tap + 512],
                                start=first,
                                stop=(kt == KT - 1 and tap == 2))
                            first = False
                    nc.vector.tensor_scalar(q2pair[:, ot, ts(j, 512)], ps[:],
                                            cb[:, ot:ot + 1], None, ADD)
                raws, den = attn_block(q2pair, j)
                if pend is not None:
                    pend()

                def mk2(raws=raws, den=den, j=j):
                    def go():
                        blk = blkp.tile([128, 2, 512], BF16, tag="c2")
                        normalize(raws, den, blk)
                        for m in range(8):
                            ps = psp.tile([128, 512], F32, tag="mm", bufs=2)
                            for kt in range(2):
                                nc.tensor.matmul(ps[:], ow[:, kt, m, :],
                                                 blk[:, kt, :],
                                                 start=(kt == 0),
                                                 stop=(kt == 1))
                            ob = obp.tile([128, 512], BF16, tag="ob")
                            nc.vector.tensor_copy(out=ob[:], in_=ps[:])
                            nc.sync.dma_start(outT_v[:, m, ts(j, 512)], ob[:])
                    return go
                pend = mk2()
            pend()

    nc.compile()
    _CACHE[key] = nc
    return nc


def prep_inputs(x, Wqkv_w, Wqkv_b, conv_w, conv_b, out_w, out_b):
    """Build the 8 per-core input maps from the full problem inputs."""
    x = np.asarray(x, np.float32)
    Wqkv_w = np.asarray(Wqkv_w, np.float32)
    Wqkv_b = np.asarray(Wqkv_b, np.float32)
    conv_w = np.asarray(conv_w, np.float32)
    conv_b = np.asarray(conv_b, np.float32)
    out_w = np.asarray(out_w, np.float32)

    scale = 1.0 / np.sqrt(DH).astype(np.float32)
    tri = (np.arange(128)[None, :] >= np.arange(128)[:, None]).astype(np.float32)
    tri2 = np.concatenate([tri, tri], axis=1).astype(ml_dtypes.bfloat16)

    in_maps = []
    for g in range(N_CORES):
        b, hg = g // 4, g % 4
        h0 = HPC * hg
        # q/k row blocks, m-tiles: [q pair0, q pair1, k pair0, k pair1]
        rows = []
        biases = []
        for blk, sc in ((0, scale), (1, 1.0)):
            for pr in range(2):
                r0 = blk * D + (h0 + 2 * pr) * DH
                rows.append(Wqkv_w[r0:r0 + 128, :] * sc)
                biases.append(Wqkv_b[r0:r0 + 128] * sc)
        wqk = np.ascontiguousarray(
            np.concatenate(rows, axis=0).T).astype(ml_dtypes.bfloat16)
        qkb = np.stack(biases, axis=0)  # [4, 128]
        c0 = CL * hg
        wv = np.ascontiguousarray(
            Wqkv_w[2 * D + c0:2 * D + c0 + CL, :].T).astype(ml_dtypes.bfloat16)
        vbb = np.ascontiguousarray(
            np.broadcast_to(Wqkv_b[2 * D + c0:2 * D + c0 + CL], (128, CL)))
        cw = np.ascontiguousarray(
            (conv_w[c0:c0 + CL, :, :] * scale).transpose(2, 1, 0)
        ).astype(ml_dtypes.bfloat16)  # [3, D, CL]
        cb = (conv_b[c0:c0 + CL] * scale).reshape(2, 128).astype(np.float32)
        owm = np.ascontiguousarray(
            out_w[:, c0:c0 + CL].T).astype(ml_dtypes.bfloat16)  # [CL, D]
        in_maps.append({
            "xT": np.ascontiguousarray(x[b].T).astype(ml_dtypes.bfloat16),
            "wqk": wqk, "wv": wv,
            "qkb": np.ascontiguousarray(qkb),
            "vbb": vbb, "cw": cw,
            "cb": np.ascontiguousarray(cb),
            "ow": owm, "tri2": tri2,
            "on1": np.ones((1, 64), ml_dtypes.bfloat16),
        })
    return in_maps


def postprocess(results, out_b):
    out_b = np.asarray(out_b, np.float32)
    out = np.empty((B, S, D), np.float32)
    for b in range(B):
        acc = np.zeros((D, S), np.float64)
        for g in GROUPS[b]:
            acc += np.asarray(results[g]["outT"], np.float64)
        out[b] = acc.T.astype(np.float32) + out_b[None, :]
    return out


def kernel(x, Wqkv_w, Wqkv_b, conv_w, conv_b, out_w, out_b):
    nc = build_kernel()
    in_maps = prep_inputs(x, Wqkv_w, Wqkv_b, conv_w, conv_b, out_w, out_b)
    res = run_bass_kernel_spmd(nc, in_maps, core_ids=list(range(N_CORES)))
    return postprocess(res.results, out_b)
